# revision 2
# baseline (speedup 1.0000x reference)
"""HRA-injected linear on 8 Trainium2 NeuronCores.

Math: reference applies r=8 sequential Householder updates to W, then y = x @ W'^T.
Compact WY form (exact): W' = W (I - V U^T) with U = normalized hra_u columns and
V computed by a tiny host-side recursion. Therefore

    y = x @ W^T - (x @ U) @ (W V)^T

The heavy term is the single big GEMM x @ W^T; the rank-8 correction is folded into
the same PSUM accumulation as one extra K=8 matmul per output tile.

Sharding: 8 cores = 4 token-groups x 2 out-feature-groups. Each core:
  x_s [2048, 4096] f32, w_s [2048, 4096] f32 -> y_s [2048, 2048] f32.
On-device: W^T built once (bf16, resident in SBUF) via PE transposes; x tiles are
converted to bf16 and transposed via the DMA xbar; main loop accumulates
32 K-tiles + 1 rank-8 correction per [128, 512] PSUM tile.
"""

import numpy as np
import ml_dtypes
from contextlib import ExitStack

import concourse.bacc as bacc
import concourse.mybir as mybir
import concourse.tile as tile
from concourse.bass_utils import run_bass_kernel_spmd
from concourse.masks import make_identity

P = 128
D = 4096          # in_features (contraction)
R = 8             # Householder rank
TOK = 8192        # 4*2048 flattened tokens
O = 4096          # out_features
TOK_GROUPS = 4
O_GROUPS = 2
TOK_S = TOK // TOK_GROUPS   # 2048 tokens per core
O_S = O // O_GROUPS         # 2048 out features per core
KT = D // P                 # 32 contraction tiles
MT = TOK_S // P             # 16 token tiles per core
NB = O_S // 512             # 4 output blocks of 512
OC = O_S // P               # 16 weight row chunks
HALF = D // 2               # 2048, chunk width for loads/converts

F32 = mybir.dt.float32
BF16 = mybir.dt.bfloat16

N_CORES = 8

_NC = None


def _build():
    nc = bacc.Bacc(None, target_bir_lowering=False)
    x_d = nc.declare_dram_parameter("x", [TOK_S, D], F32, isOutput=False)
    w_d = nc.declare_dram_parameter("w", [O_S, D], F32, isOutput=False)
    u_d = nc.declare_dram_parameter("u", [P, KT, R], BF16, isOutput=False)
    vn_d = nc.declare_dram_parameter("vn", [P, KT, R], BF16, isOutput=False)
    y_d = nc.declare_dram_parameter("out", [TOK_S, O_S], F32, isOutput=True)

    with tile.TileContext(nc) as tc, ExitStack() as ctx:
        const = ctx.enter_context(tc.tile_pool(name="const", bufs=1))
        wt_pool = ctx.enter_context(tc.tile_pool(name="wtp", bufs=1))
        inp = ctx.enter_context(tc.tile_pool(name="inp", bufs=2))
        bfp = ctx.enter_context(tc.tile_pool(name="bfp", bufs=2))
        xtp = ctx.enter_context(tc.tile_pool(name="xtp", bufs=2))
        ysb = ctx.enter_context(tc.tile_pool(name="ysb", bufs=4))
        smal = ctx.enter_context(tc.tile_pool(name="smal", bufs=2))
        psum = ctx.enter_context(tc.tile_pool(name="psum", bufs=1, space="PSUM"))

        ident = const.tile([P, P], BF16)
        make_identity(nc, ident)
        u_sb = const.tile([P, KT, R], BF16)
        nc.sync.dma_start(out=u_sb, in_=u_d[:])
        vn_sb = const.tile([P, KT, R], BF16)
        nc.sync.dma_start(out=vn_sb, in_=vn_d[:])
        wv_sb = const.tile([R, O_S], BF16)

        # resident W^T, bf16: wt[p, k, o] = W'[o, k*128+p] (pre-update W here)
        wt = wt_pool.tile([P, KT, O_S], BF16)

        # ---- Phase 1: build W^T via PE transposes ----
        for c in range(OC):            # 16 chunks of 128 weight rows
            for h in range(2):         # two halves of d
                w_in = inp.tile([P, HALF], F32, tag="in")
                nc.sync.dma_start(
                    out=w_in, in_=w_d[c * P:(c + 1) * P, h * HALF:(h + 1) * HALF]
                )
                w_bf = bfp.tile([P, HALF], BF16, tag="bf")
                nc.vector.tensor_copy(out=w_bf, in_=w_in)
                for g in range(4):     # groups of 4 k-subtiles share a psum bank
                    ps_t = psum.tile([P, 4, P], BF16, tag="stage", bufs=2)
                    for j in range(4):
                        nc.tensor.transpose(
                            ps_t[:, j, :], w_bf[:, (g * 4 + j) * P:(g * 4 + j + 1) * P],
                            ident,
                        )
                    k0 = h * 16 + g * 4
                    nc.vector.tensor_copy(
                        out=wt[:, k0:k0 + 4, c * P:(c + 1) * P], in_=ps_t
                    )

        # ---- Phase 2: WVt_neg [8, O_S] = (-V)^T @ W^T ----
        for nb in range(NB):
            ps_wv = psum.tile([R, 512], F32, tag="pwv", bufs=1)
            for k in range(KT):
                nc.tensor.matmul(
                    ps_wv, vn_sb[:, k, :], wt[:, k, nb * 512:(nb + 1) * 512],
                    start=(k == 0), stop=(k == KT - 1),
                )
            nc.vector.tensor_copy(out=wv_sb[:, nb * 512:(nb + 1) * 512], in_=ps_wv)

        # ---- Phase 3: main loop over token tiles ----
        for m in range(MT):
            xt = xtp.tile([P, KT, P], BF16, tag="xt")
            for h in range(2):
                x_in = inp.tile([P, HALF], F32, tag="in")
                nc.sync.dma_start(
                    out=x_in, in_=x_d[m * P:(m + 1) * P, h * HALF:(h + 1) * HALF]
                )
                x_bf = bfp.tile([P, HALF], BF16, tag="bf")
                nc.vector.tensor_copy(out=x_bf, in_=x_in)
                # xbar transpose: xt[p, k, t] = x_bf[t, k*128+p]
                nc.sync.dma_start(out=xt[:, h * 16:(h + 1) * 16, :], in_=x_bf,
                                  transpose=True)

            # P^T = U^T x^T  [8, 128] for the rank-8 correction
            ps_p = psum.tile([R, P], F32, tag="pp", bufs=1)
            for k in range(KT):
                nc.tensor.matmul(ps_p, u_sb[:, k, :], xt[:, k, :],
                                 start=(k == 0), stop=(k == KT - 1))
            pt_sb = smal.tile([R, P], BF16, tag="pt")
            nc.vector.tensor_copy(out=pt_sb, in_=ps_p)

            ps_y = [
                psum.tile([P, 512], F32, tag=f"py{nb}", bufs=1, name=f"ps_y{nb}")
                for nb in range(NB)
            ]
            for k in range(KT):
                for nb in range(NB):
                    nc.tensor.matmul(
                        ps_y[nb], xt[:, k, :], wt[:, k, nb * 512:(nb + 1) * 512],
                        start=(k == 0), stop=False,
                    )
            for nb in range(NB):
                # correction: y += P @ WVt_neg  (K=8)
                nc.tensor.matmul(
                    ps_y[nb], pt_sb, wv_sb[:, nb * 512:(nb + 1) * 512],
                    start=False, stop=True,
                )
                y_t = ysb.tile([P, 512], F32, tag="y")
                nc.vector.tensor_copy(out=y_t, in_=ps_y[nb])
                nc.sync.dma_start(
                    out=y_d[m * P:(m + 1) * P, nb * 512:(nb + 1) * 512], in_=y_t
                )

    nc.compile()
    return nc


def _get_nc():
    global _NC
    if _NC is None:
        _NC = _build()
    return _NC


def _host_prep(hra_u):
    """Normalize u columns and compute V of the compact WY form, in float64."""
    u = hra_u.astype(np.float64)
    u = u / np.linalg.norm(u, axis=0, keepdims=True)        # [D, R]
    v = np.zeros_like(u)
    for k_ in range(R):
        acc = u[:, k_].copy()
        for j in range(k_):
            acc -= v[:, j] * np.dot(u[:, j], u[:, k_])
        v[:, k_] = 2.0 * acc
    # chunked layouts [P, KT, R]: t[p, k, r] = m[k*128+p, r]
    def chunk(m):
        return np.ascontiguousarray(
            m.reshape(KT, P, R).transpose(1, 0, 2)
        ).astype(ml_dtypes.bfloat16)
    return chunk(u), chunk(-v)


def kernel(x, weight, hra_u):
    nc = _get_nc()
    u_c, vn_c = _host_prep(hra_u)
    xf = np.ascontiguousarray(x.reshape(TOK, D)).astype(np.float32)

    in_maps = []
    for core in range(N_CORES):
        a, b = core // O_GROUPS, core % O_GROUPS
        in_maps.append({
            "x": np.ascontiguousarray(xf[a * TOK_S:(a + 1) * TOK_S]),
            "w": np.ascontiguousarray(weight[b * O_S:(b + 1) * O_S]).astype(np.float32),
            "u": u_c,
            "vn": vn_c,
        })

    res = run_bass_kernel_spmd(nc, in_maps, core_ids=list(range(N_CORES))).results

    y = np.empty((TOK, O), dtype=np.float32)
    for core in range(N_CORES):
        a, b = core // O_GROUPS, core % O_GROUPS
        y[a * TOK_S:(a + 1) * TOK_S, b * O_S:(b + 1) * O_S] = res[core]["out"]
    return y.reshape(x.shape[0], x.shape[1], O)
